# revision 7
# baseline (speedup 1.0000x reference)
"""Trainium2 Bass kernel for nn_MultiHeadAttention_32031866093611.

Sharding: pure data parallel — batch b -> NeuronCore b (B == n_cores == 8).
Weights replicated. No collectives.

Per-core program (batch b, S=1024, D=1024, H=16, DK=64), matmuls fp32r except
the PV stage which runs bf16 (exp output + v tiles), all PSUM accum fp32:

  qT[c] = (Wq[:, c*128:+128]).T @ xT + bq  -> [128 d', 1024 s]  (ACT Identity+bias)
  kT[c] = (Wk[:, c*128:+128]).T @ xT       -> [128 d', 1024 s]  (ACT Copy; bk is
          dropped exactly: softmax over k is invariant to the q·bk term)
  v[sc] = (xT[:, sc*128:+128]).T @ Wv      -> [128 s, 16, 64+1] bf16 (ones col;
          bv is folded on host into bo_eff = bv @ Wo + bo, exact since sum(p)=1)
  per head h (c=h//2, r=h%2*64), kc DESCENDING 7..0 with width W[kc] =
  max(max_prefix, (kc+1)*128)  (cols >= W[kc] are masked on every core):
    sT[kc] = kT[c][r:r+64, kc*128:+128].T @ qT[c][r:r+64, 0:W]   # [128 k, W q]
    eT[kc] = exp(sT[kc]) -> bf16                                  # ACT
    eT[kc][:, kc*128:W] *= mask (bf16 0/1, host-built, 4x DVE mode)
    outT  += v[kc][:, h, :].T @ eT[kc][:, 0:W]   # [65, W]; row 64 = denom
  attnT[c][r:r+64, :] = outT[0:64, :] * bcast(1/outT[64, :])
  out[sc] = (attnT[.][:, sc*128:+128]).T @ Wo + bo_eff -> [128 s, 1024 d] -> DRAM

Schedule (single in-order PE stream, PE is the binding engine at ~92% of the
kernel): per-chunk W loads let the first q/k projection start ~13us in; the v
projection tiles (descending sc, matching the descending-kc PV accumulation)
are woven between head 0's score tiles; q/k projections for chunk c+1 are
woven into head 2c+1's stream; o_proj chunk k fires two heads after head 2k+1
retires. Scores/exp/mask/PV share two PSUM score slots with the woven
projection psums (the PE never holds more than two `pp` tiles at once); PV
lags scores by 4 tiles so the in-order PE never waits on a just-issued exp.
"""

import numpy as np
import ml_dtypes

import concourse.bass as bass
import concourse.mybir as mybir
import concourse.tile as tile
from concourse import bacc
from concourse.bass_utils import run_bass_kernel_spmd

B, S, D, H = 8, 1024, 1024, 16
DK = D // H  # 64
P = 128
NCH = S // P  # 8
NCORES = 8
F32R = mybir.dt.float32r
F32 = mybir.dt.float32
BF16 = mybir.dt.bfloat16
EXP = mybir.ActivationFunctionType.Exp
IDENT = mybir.ActivationFunctionType.Identity
COPY = mybir.ActivationFunctionType.Copy
HALF = 512  # fp32 moving-operand max / one PSUM bank of fp32

_CACHED = {}


def _widths(pmax):
    """Score/exp/PV column widths per k-tile; W[7] == 1024 always."""
    return [max(pmax, (kc + 1) * P) for kc in range(NCH)]


def build_nc(pmax):
    Wd = _widths(pmax)
    mskw = [Wd[kc] - kc * P for kc in range(NCH)]
    moff = [0]
    for w in mskw[:-1]:
        moff.append(moff[-1] + w)
    msk_total = sum(mskw)
    # last-executed writer of PSUM bank 1 in the descending-kc PV group
    last_b1 = min(kc for kc in range(NCH) if Wd[kc] > HALF)

    nc = bacc.Bacc("TRN2", target_bir_lowering=False, debug=False, num_devices=NCORES)

    xt_d = nc.dram_tensor("xt", [D, S], F32R, kind="ExternalInput").ap()
    wq_d = nc.dram_tensor("wq", [D, D], F32R, kind="ExternalInput").ap()
    wk_d = nc.dram_tensor("wk", [D, D], F32R, kind="ExternalInput").ap()
    wv_d = nc.dram_tensor("wv", [D, D], F32R, kind="ExternalInput").ap()
    wo_d = nc.dram_tensor("wo", [D, D], F32R, kind="ExternalInput").ap()
    bq_d = nc.dram_tensor("bq8", [P, NCH], F32, kind="ExternalInput").ap()
    ones_d = nc.dram_tensor("ones2d", [P, P], F32R, kind="ExternalInput").ap()
    boe_d = nc.dram_tensor("boe", [P, D], F32, kind="ExternalInput").ap()
    msk_d = nc.dram_tensor("mskb", [P, msk_total], BF16, kind="ExternalInput").ap()
    out_d = nc.dram_tensor("out", [S, D], F32, kind="ExternalOutput").ap()

    with tile.TileContext(nc) as tc:
        with (
            tc.tile_pool(name="w", bufs=18) as wpool,        # wv + wo [P,512] strips
            tc.tile_pool(name="wqk", bufs=32) as wqkpool,    # per-chunk [P,P] q/k blocks
            tc.tile_pool(name="big", bufs=2) as bigpool,     # xT
            tc.tile_pool(name="atn", bufs=2) as atnpool,     # attn (own pool: xT is
            tc.tile_pool(name="qk", bufs=3) as qkpool,       #  still live at norm(0))
            tc.tile_pool(name="v", bufs=8) as vpool,
            tc.tile_pool(name="cst", bufs=1) as cstpool,
            tc.tile_pool(name="exp", bufs=5) as exppool,
            tc.tile_pool(name="rcp", bufs=2) as rcppool,
            tc.tile_pool(name="rbc", bufs=2) as rbcpool,
            tc.tile_pool(name="osb", bufs=2) as osbpool,
            tc.tile_pool(name="pp", bufs=2, space="PSUM") as pp,
            tc.tile_pool(name="po", bufs=2, space="PSUM") as po,
        ):
            # ---- DMA front: x + chunk-0 W blocks first for a fast PE start ----
            ones2d = cstpool.tile([P, P], F32R, tag="ones2d")
            nc.sync.dma_start(ones2d[:], ones_d[:])
            bq8 = cstpool.tile([P, NCH], F32, tag="bq8")
            nc.sync.dma_start(bq8[:], bq_d[:])
            xtq = [
                bigpool.tile([P, 4, S], F32R, tag="big", name=f"xtq_{g}")
                for g in range(2)
            ]

            def load_wchunk(nm, w_dram, c):
                """8 [128,128] blocks of W columns [c*128, (c+1)*128)."""
                ts = [
                    wqkpool.tile([P, P], F32R, tag="wqk", name=f"{nm}{c}_{dc}")
                    for dc in range(NCH)
                ]
                for dc in range(NCH):
                    nc.sync.dma_start(
                        ts[dc][:], w_dram[dc * P : (dc + 1) * P, c * P : (c + 1) * P]
                    )
                return ts

            def whalf(nm, w_dram, hf):
                """8 half-strips [128, 512] of W columns [hf*512, (hf+1)*512)."""
                ts = [
                    wpool.tile([P, HALF], F32R, tag="w", name=f"{nm}{hf}_{dc}")
                    for dc in range(NCH)
                ]
                sl = slice(hf * HALF, (hf + 1) * HALF)
                for dc in range(NCH):
                    nc.sync.dma_start(ts[dc][:], w_dram[dc * P : (dc + 1) * P, sl])
                return ts

            for dc in range(NCH):
                nc.sync.dma_start(
                    xtq[dc // 4][:, dc % 4, 0:HALF],
                    xt_d[dc * P : (dc + 1) * P, 0:HALF],
                )
            wqc = {0: load_wchunk("wq", wq_d, 0)}
            for dc in range(NCH):
                nc.sync.dma_start(
                    xtq[dc // 4][:, dc % 4, HALF:S],
                    xt_d[dc * P : (dc + 1) * P, HALF:S],
                )
            wkc = {0: load_wchunk("wk", wk_d, 0)}
            xt = [xtq[dc // 4][:, dc % 4, :] for dc in range(NCH)]
            msk = cstpool.tile([P, msk_total], BF16, tag="msk")
            nc.sync.dma_start(msk[:], msk_d[:])
            vh = [whalf("wv", wv_d, 0), whalf("wv", wv_d, 1)]
            wqc[1] = load_wchunk("wq", wq_d, 1)
            wkc[1] = load_wchunk("wk", wk_d, 1)

            # PE warm-up: throwaway matmuls on the first-arriving tiny tile
            # keep the HAM clock-gate hot while x/Wq stream in; alternate PSUM
            # banks so consecutive start/stop groups don't serialize.
            wps = pp.tile([P, S], F32, tag="pp", name="warmup_ps")
            for wi in range(14):
                off = (wi % 2) * HALF
                nc.tensor.matmul(
                    wps[:, off : off + P], ones2d[:], ones2d[:], start=True, stop=True
                )

            qT, kT = {}, {}

            def proj_qk(c, kind):
                wts = (wqc if kind == "q" else wkc)[c]
                pss = pp.tile([P, S], F32, tag="pp", name=f"ps_{kind}{c}")
                for j in range(2):
                    sl = slice(j * HALF, (j + 1) * HALF)
                    for dc in range(NCH):
                        nc.tensor.matmul(
                            pss[:, sl],
                            wts[dc][:],
                            xt[dc][:, sl],
                            start=(dc == 0),
                            stop=(dc == NCH - 1),
                        )
                o = qkpool.tile(
                    [P, S], F32R, tag="qT" if kind == "q" else "kT",
                    name=f"{kind}T_{c}",
                )
                if kind == "q":
                    nc.scalar.activation(o[:], pss[:], IDENT, bias=bq8[:, c : c + 1])
                else:
                    nc.scalar.activation(o[:], pss[:], COPY)
                (qT if kind == "q" else kT)[c] = o

            vtiles = {}

            def proj_v(sc):
                ps = pp.tile([P, S], F32, tag="pp", name=f"ps_v{sc}")
                for j in range(2):
                    sl = slice(j * HALF, (j + 1) * HALF)
                    for dc in range(NCH):
                        nc.tensor.matmul(
                            ps[:, sl],
                            xt[dc][:, sc * P : (sc + 1) * P],
                            vh[j][dc][:],
                            start=(dc == 0),
                            stop=(dc == NCH - 1),
                        )
                vt = vpool.tile([P, H, DK + 1], BF16, tag="v", name=f"vt_{sc}")
                nc.scalar.activation(
                    vt[:, :, 0:DK], ps[:].rearrange("p (h d) -> p h d", h=H), COPY
                )
                nc.vector.memset(vt[:, :, DK : DK + 1], 1.0)
                vtiles[sc] = vt

            attn = [None, None]
            oh = [None, None]
            boe = cstpool.tile([P, D], F32, tag="boe")

            def emit_scores_exp(h, kc):
                """scores on PE, exp->bf16 on ACT, 0/1 bf16 mask mult on DVE."""
                c, r = h // 2, (h % 2) * DK
                Wc = Wd[kc]
                pss = pp.tile([P, S], F32, tag="pp", name=f"pss_{h}_{kc}")
                lhs = kT[c][r : r + DK, kc * P : (kc + 1) * P]
                p0 = min(Wc, HALF)
                nc.tensor.matmul(
                    pss[:, 0:p0], lhs, qT[c][r : r + DK, 0:p0], start=True, stop=True
                )
                if Wc > HALF:
                    nc.tensor.matmul(
                        pss[:, HALF:Wc],
                        lhs,
                        qT[c][r : r + DK, HALF:Wc],
                        start=True,
                        stop=True,
                    )
                et = exppool.tile([P, S], BF16, tag="exp", name=f"et_{h}_{kc}")
                nc.scalar.activation(et[:, 0:Wc], pss[:, 0:Wc], EXP)
                off = moff[kc]
                w = Wc - kc * P
                nc.vector.tensor_mul(
                    et[:, kc * P : Wc], et[:, kc * P : Wc], msk[:, off : off + w]
                )
                return et

            def emit_pv(h, kc, pso, et):
                Wc = Wd[kc]
                vs = vtiles[kc][:, h, :]
                p0 = min(Wc, HALF)
                nc.tensor.matmul(
                    pso[0 : DK + 1, 0:p0],
                    vs,
                    et[:, 0:p0],
                    start=(kc == NCH - 1),
                    stop=(kc == 0),
                )
                if Wc > HALF:
                    nc.tensor.matmul(
                        pso[0 : DK + 1, HALF:Wc],
                        vs,
                        et[:, HALF:Wc],
                        start=(kc == NCH - 1),
                        stop=(kc == last_b1),
                    )

            def emit_norm(h, pso):
                """Recip on DVE (PSUM->SBUF), bcast on Pool, muls on DVE.

                attn[g][e*64+d, cc, h*64+u] = O_h[u*16 + 2*(4g+cc) + e, d]/denom
                """
                rcp = rcppool.tile([1, S], F32, tag="rcp", name=f"rcp_{h}")
                nc.vector.reciprocal(rcp[:], pso[DK : DK + 1, :])
                rbc = rbcpool.tile([DK, S], F32, tag="rbc", name=f"rbc_{h}")
                nc.gpsimd.partition_broadcast(rbc[:], rcp[:])
                src = pso[0:DK, :].rearrange("d (u j) -> d j u", j=16)
                rbs = rbc[:].rearrange("d (u j) -> d j u", j=16)
                for g in range(2):
                    if attn[g] is None:
                        attn[g] = atnpool.tile(
                            [P, 4, S], F32R, tag="atn", name=f"attnq_{g}"
                        )
                    for e in range(2):
                        jsl = slice(8 * g + e, 8 * (g + 1), 2)
                        nc.vector.tensor_mul(
                            attn[g][e * DK : (e + 1) * DK, :, h * DK : (h + 1) * DK],
                            src[:, jsl, :],
                            rbs[:, jsl, :],
                        )

            def emit_oproj(sc):
                ps = po.tile([P, S], F32, tag="po", name=f"psf_{sc}")
                for j in range(2):
                    sl = slice(j * HALF, (j + 1) * HALF)
                    for cc in range(NCH):
                        nc.tensor.matmul(
                            ps[:, sl],
                            attn[cc // 4][:, cc % 4, sc * P : (sc + 1) * P],
                            oh[j][cc][:],
                            start=(cc == 0),
                            stop=(cc == NCH - 1),
                        )
                ot = osbpool.tile([P, S], F32, tag="osb", name=f"ot_{sc}")
                nc.vector.tensor_add(ot[:], ps[:], boe[:])
                nc.sync.dma_start(out_d[sc * P : (sc + 1) * P, :], ot[:])

            # ---- PE filler micro-ops: 4 matmuls each, woven between tiles ----
            def qk_fillers(c):
                """8 fillers building qT[c] then kT[c] (one j-half group of 4
                contraction steps each); pss allocated lazily per kind."""
                cell = {}

                def mk(i):
                    def run():
                        kind = "q" if i < 4 else "k"
                        j = (i % 4) // 2
                        dc0 = (i % 2) * 4
                        if i % 4 == 0:
                            cell[kind] = pp.tile(
                                [P, S], F32, tag="pp", name=f"ps_{kind}{c}"
                            )
                        ps = cell[kind]
                        wts = (wqc if kind == "q" else wkc)[c]
                        sl = slice(j * HALF, (j + 1) * HALF)
                        for dc in range(dc0, dc0 + 4):
                            nc.tensor.matmul(
                                ps[:, sl],
                                wts[dc][:],
                                xt[dc][:, sl],
                                start=(dc == 0),
                                stop=(dc == NCH - 1),
                            )
                        if i == 3:
                            o = qkpool.tile([P, S], F32R, tag="qT", name=f"qT_{c}")
                            nc.scalar.activation(
                                o[:], ps[:], IDENT, bias=bq8[:, c : c + 1]
                            )
                            qT[c] = o
                        elif i == 7:
                            o = qkpool.tile([P, S], F32R, tag="kT", name=f"kT_{c}")
                            nc.scalar.activation(o[:], ps[:], COPY)
                            kT[c] = o

                    return run

                return [mk(i) for i in range(8)]

            def oproj_fillers(sc):
                """4 fillers (j-half x cc-half) + bias/DMA on the last one."""
                cell = {}

                def mk(i):
                    def run():
                        if i == 0:
                            cell["ps"] = po.tile([P, S], F32, tag="po", name=f"psf_{sc}")
                        ps = cell["ps"]
                        j = i // 2
                        sl = slice(j * HALF, (j + 1) * HALF)
                        for cc in range((i % 2) * 4, (i % 2) * 4 + 4):
                            nc.tensor.matmul(
                                ps[:, sl],
                                attn[cc // 4][:, cc % 4, sc * P : (sc + 1) * P],
                                oh[j][cc][:],
                                start=(cc == 0),
                                stop=(cc == NCH - 1),
                            )
                        if i == 3:
                            ot = osbpool.tile([P, S], F32, tag="osb", name=f"ot_{sc}")
                            nc.vector.tensor_add(ot[:], ps[:], boe[:])
                            nc.sync.dma_start(out_d[sc * P : (sc + 1) * P, :], ot[:])

                    return run

                return [mk(i) for i in range(4)]

            # ---- the single interleaved stream ----
            from collections import deque

            pend = deque()

            def pop_pv():
                ph, pkc, ppso, pet = pend.popleft()
                emit_pv(ph, pkc, ppso, pet)
                if pkc == 0:
                    emit_norm(ph, ppso)

            proj_qk(0, "q")
            proj_qk(0, "k")

            for h in range(H):
                c = h // 2
                pso_cur = po.tile([P, S], F32, tag="po", name=f"pso_{h}")
                # W chunk prefetch two heads ahead of use (chunk m feeds the
                # qk fillers hosted on head 2(m-1))
                if h in (1, 2, 4, 6, 8, 10):
                    m = 2 if h == 1 else h // 2 + 2
                    if m <= NCH - 1:
                        wqc[m] = load_wchunk("wq", wq_d, m)
                        wkc[m] = load_wchunk("wk", wk_d, m)
                # filler assignment per head:
                #  h0: v tiles (descending, matches PV kc order)
                #  h1: qk(1) spread over all 8 tiles
                #  even h>=2: qk(c+1) spread over all 8 tiles
                #  odd h>=3: o_proj((h-3)//2) on tiles kc=3..0 — the psf PSUM
                #   slot only frees up at the norm(h-1) pop (tile kc=4)
                fills = {}
                if h == 1:
                    fl = qk_fillers(1)
                    fills = {7 - i: fl[i] for i in range(8)}
                elif h >= 2 and h % 2 == 0 and c + 1 <= NCH - 1:
                    fl = qk_fillers(c + 1)
                    fills = {7 - i: fl[i] for i in range(8)}
                elif h >= 3 and h % 2 == 1:
                    fl = oproj_fillers((h - 3) // 2)
                    fills = {3 - i: fl[i] for i in range(4)}
                for kc in range(NCH - 1, -1, -1):
                    et = emit_scores_exp(h, kc)
                    if len(pend) >= 4:
                        pop_pv()
                    pend.append((h, kc, pso_cur, et))
                    if h == 0:
                        # v tiles woven into head 0, descending to match PV order
                        proj_v(kc)
                        if kc == 0:
                            # Wo strips + boe: DMA queue position after wv's
                            # last use frees wpool slots
                            oh[0] = whalf("wo", wo_d, 0)
                            oh[1] = whalf("wo", wo_d, 1)
                            nc.sync.dma_start(boe[:], boe_d[:])
                    elif kc in fills:
                        fills[kc]()
            while len(pend) > 1:
                pop_pv()
            ph, pkc, ppso, pet = pend.popleft()
            emit_pv(ph, pkc, ppso, pet)
            emit_norm(ph, ppso)
            emit_oproj(NCH - 1)

    nc.compile()
    return nc


def _host_mask(prefix_b, pmax):
    """Per-core multiplicative 0/1 bf16 mask over cols [kc*128, W[kc])."""
    Wd = _widths(pmax)
    i = np.arange(P)[:, None]
    segs = []
    for kc in range(NCH):
        q = np.arange(kc * P, Wd[kc])[None, :]
        k = kc * P + i
        allowed = (q < prefix_b) | (k >= q)
        segs.append(allowed.astype(ml_dtypes.bfloat16))
    return np.concatenate(segs, axis=1)


def kernel(x, prefix, Wq, bq, Wk, bk, Wv, bv, Wo, bo, _trace=False):
    x = np.asarray(x, dtype=np.float32)
    prefix = np.asarray(prefix)
    Wq, Wk, Wv, Wo = (
        np.ascontiguousarray(np.asarray(w, np.float32)) for w in (Wq, Wk, Wv, Wo)
    )
    pmax = int(prefix.max())
    # Exact folds: softmax_k[(q+bq)·(k+bk)] == softmax_k[(q+bq)·k]  (q·bk and
    # bq·bk are constant over k); out = attn@Wo + (bv@Wo + bo) since sum(p)=1.
    boe = (
        np.asarray(bv, np.float64) @ np.asarray(Wo, np.float64) + np.asarray(bo)
    ).astype(np.float32)
    boe_bc = np.broadcast_to(boe.reshape(1, D), (P, D)).copy()
    bq8 = np.asarray(bq, np.float32).reshape(NCH, P).T.copy()  # [128, 8] cols
    ones2d = np.ones((P, P), dtype=np.float32)

    if pmax not in _CACHED:
        _CACHED[pmax] = build_nc(pmax)
    nc = _CACHED[pmax]

    in_maps = []
    for b in range(B):
        in_maps.append(
            {
                "xt": np.ascontiguousarray(x[b].T),
                "wq": Wq, "wk": Wk, "wv": Wv, "wo": Wo,
                "bq8": bq8, "boe": boe_bc, "ones2d": ones2d,
                "mskb": _host_mask(int(prefix[b]), pmax),
            }
        )

    res = run_bass_kernel_spmd(nc, in_maps, core_ids=list(range(NCORES)), trace=_trace)
    out = np.stack([res.results[b]["out"] for b in range(B)], axis=0)
    if _trace:
        return out, res
    return out


# revision 16
# speedup vs baseline: 1.0003x; 1.0003x over previous
"""Trainium2 Bass kernel for nn_MultiHeadAttention_32031866093611.

Sharding: pure data parallel — batch b -> NeuronCore b (B == n_cores == 8).
Weights replicated. No collectives.

Per-core program (batch b, S=1024, D=1024, H=16, DK=64), matmuls fp32r except
the PV stage which runs bf16 (exp output + v tiles), all PSUM accum fp32:

  qT[c] = (Wq[:, c*128:+128]).T @ xT + bq  -> [128 d', 1024 s]  (ACT Identity+bias)
  kT[c] = (Wk[:, c*128:+128]).T @ xT       -> [128 d', 1024 s]  (ACT Copy; bk is
          dropped exactly: softmax over k is invariant to the q·bk term)
  v[sc] = (xT[:, sc*128:+128]).T @ Wv      -> [128 s, 16, 64+1] bf16 (ones col;
          bv is folded on host into bo_eff = bv @ Wo + bo, exact since sum(p)=1)
  per head h (c=h//2, r=h%2*64), kc DESCENDING 7..0 with width W[kc] =
  max(max_prefix, (kc+1)*128)  (cols >= W[kc] are masked on every core):
    sT[kc] = kT[c][r:r+64, kc*128:+128].T @ qT[c][r:r+64, 0:W]   # [128 k, W q]
    eT[kc] = exp(sT[kc]) -> bf16                                  # ACT
    eT[kc][:, kc*128:W] *= mask (bf16 0/1, host-built, 4x DVE mode)
    outT  += v[kc][:, h, :].T @ eT[kc][:, 0:W]   # [65, W]; row 64 = denom
  attnT[c][r:r+64, :] = outT[0:64, :] * bcast(1/outT[64, :])
  out[sc] = (attnT[.][:, sc*128:+128]).T @ Wo + bo_eff -> [128 s, 1024 d] -> DRAM

Schedule (single in-order PE stream, PE is the binding engine at ~92% of the
kernel): per-chunk W loads let the first q/k projection start ~13us in; the v
projection tiles (descending sc, matching the descending-kc PV accumulation)
are woven between head 0's score tiles; q/k projections for chunk c+1 are
woven into head 2c+1's stream; o_proj chunk k fires two heads after head 2k+1
retires. Scores/exp/mask/PV share two PSUM score slots with the woven
projection psums (the PE never holds more than two `pp` tiles at once); PV
lags scores by 4 tiles so the in-order PE never waits on a just-issued exp.
"""

import numpy as np
import ml_dtypes

import concourse.bass as bass
import concourse.mybir as mybir
import concourse.tile as tile
from concourse import bacc
from concourse.bass_utils import run_bass_kernel_spmd

B, S, D, H = 8, 1024, 1024, 16
DK = D // H  # 64
P = 128
NCH = S // P  # 8
NCORES = 8
F32R = mybir.dt.float32r
F32 = mybir.dt.float32
BF16 = mybir.dt.bfloat16
EXP = mybir.ActivationFunctionType.Exp
IDENT = mybir.ActivationFunctionType.Identity
COPY = mybir.ActivationFunctionType.Copy
HALF = 512  # fp32 moving-operand max / one PSUM bank of fp32

_CACHED = {}


def _widths(pmax):
    """Score/exp/PV column widths per k-tile; W[7] == 1024 always."""
    return [max(pmax, (kc + 1) * P) for kc in range(NCH)]


def build_nc(pmax):
    Wd = _widths(pmax)
    mskw = [Wd[kc] - kc * P for kc in range(NCH)]
    moff = [0]
    for w in mskw[:-1]:
        moff.append(moff[-1] + w)
    msk_total = sum(mskw)
    # last-executed writer of PSUM bank 1 in the descending-kc PV group
    last_b1 = min(kc for kc in range(NCH) if Wd[kc] > HALF)

    nc = bacc.Bacc("TRN2", target_bir_lowering=False, debug=False, num_devices=NCORES)

    xt_d = nc.dram_tensor("xt", [D, S], F32R, kind="ExternalInput").ap()
    wq_d = nc.dram_tensor("wq", [D, D], F32R, kind="ExternalInput").ap()
    wk_d = nc.dram_tensor("wk", [D, D], F32R, kind="ExternalInput").ap()
    wv_d = nc.dram_tensor("wv", [D, D], BF16, kind="ExternalInput").ap()
    wo_d = nc.dram_tensor("wo", [D, D], BF16, kind="ExternalInput").ap()
    bq_d = nc.dram_tensor("bq8", [P, NCH], F32, kind="ExternalInput").ap()
    ones_d = nc.dram_tensor("ones2d", [P, P], F32R, kind="ExternalInput").ap()
    boe_d = nc.dram_tensor("boe", [P, D], F32, kind="ExternalInput").ap()
    msk_d = nc.dram_tensor("mskb", [P, msk_total], BF16, kind="ExternalInput").ap()
    out_d = nc.dram_tensor("out", [S, D], F32, kind="ExternalOutput").ap()

    with tile.TileContext(nc) as tc:
        with (
            tc.tile_pool(name="w", bufs=18) as wpool,        # wv + wo [P,512] strips
            tc.tile_pool(name="wqk", bufs=32) as wqkpool,    # per-chunk [P,P] q/k blocks
            tc.tile_pool(name="big", bufs=2) as bigpool,     # xT
            tc.tile_pool(name="atn", bufs=2) as atnpool,     # attn (own pool: xT is
            tc.tile_pool(name="qk", bufs=3) as qkpool,       #  still live at norm(0))
            tc.tile_pool(name="v", bufs=8) as vpool,
            tc.tile_pool(name="cst", bufs=1) as cstpool,
            tc.tile_pool(name="exp", bufs=5) as exppool,
            tc.tile_pool(name="rcp", bufs=2) as rcppool,
            tc.tile_pool(name="rbc", bufs=2) as rbcpool,
            tc.tile_pool(name="osb", bufs=2) as osbpool,
            tc.tile_pool(name="pp", bufs=2, space="PSUM") as pp,
            tc.tile_pool(name="po", bufs=2, space="PSUM") as po,
        ):
            # ---- DMA front: x + chunk-0 W blocks first for a fast PE start ----
            ones2d = cstpool.tile([P, P], F32R, tag="ones2d")
            nc.sync.dma_start(ones2d[:], ones_d[:])
            bq8 = cstpool.tile([P, NCH], F32, tag="bq8")
            nc.sync.dma_start(bq8[:], bq_d[:])
            xtq = [
                bigpool.tile([P, 4, S], F32R, tag="big", name=f"xtq_{g}")
                for g in range(2)
            ]

            def load_wchunk(nm, w_dram, c):
                """8 [128,128] blocks of W columns [c*128, (c+1)*128)."""
                ts = [
                    wqkpool.tile([P, P], F32R, tag="wqk", name=f"{nm}{c}_{dc}")
                    for dc in range(NCH)
                ]
                for dc in range(NCH):
                    nc.sync.dma_start(
                        ts[dc][:], w_dram[dc * P : (dc + 1) * P, c * P : (c + 1) * P]
                    )
                return ts

            def whalf(nm, w_dram, hf, dt=F32R):
                """8 half-strips [128, 512] of W columns [hf*512, (hf+1)*512)."""
                ts = [
                    wpool.tile([P, HALF], dt, tag="w", name=f"{nm}{hf}_{dc}")
                    for dc in range(NCH)
                ]
                sl = slice(hf * HALF, (hf + 1) * HALF)
                for dc in range(NCH):
                    nc.sync.dma_start(ts[dc][:], w_dram[dc * P : (dc + 1) * P, sl])
                return ts

            # x/W chunk-0 DMAs interleaved per dc so the first projection's
            # accumulation streams with the DMA instead of after it
            wqc = {
                0: [
                    wqkpool.tile([P, P], F32R, tag="wqk", name=f"wq0_{dc}")
                    for dc in range(NCH)
                ]
            }
            for dc in range(NCH):
                nc.sync.dma_start(
                    xtq[dc // 4][:, dc % 4, 0:HALF],
                    xt_d[dc * P : (dc + 1) * P, 0:HALF],
                )
                nc.sync.dma_start(wqc[0][dc][:], wq_d[dc * P : (dc + 1) * P, 0:P])
            wkc = {
                0: [
                    wqkpool.tile([P, P], F32R, tag="wqk", name=f"wk0_{dc}")
                    for dc in range(NCH)
                ]
            }
            for dc in range(NCH):
                nc.sync.dma_start(
                    xtq[dc // 4][:, dc % 4, HALF:S],
                    xt_d[dc * P : (dc + 1) * P, HALF:S],
                )
                nc.sync.dma_start(wkc[0][dc][:], wk_d[dc * P : (dc + 1) * P, 0:P])
            xt = [xtq[dc // 4][:, dc % 4, :] for dc in range(NCH)]
            # bf16 copy of xT: stationary operand of the v projection (the
            # backend requires matmul operand dtypes to match; wv is bf16)
            xtq16 = [
                bigpool.tile([P, 4, S], BF16, tag="xt16", name=f"xtq16_{g}")
                for g in range(2)
            ]
            xt16 = [xtq16[dc // 4][:, dc % 4, :] for dc in range(NCH)]
            vh = [whalf("wv", wv_d, 0, BF16)]
            msk = cstpool.tile([P, msk_total], BF16, tag="msk")
            nc.sync.dma_start(msk[:], msk_d[:])
            vh.append(whalf("wv", wv_d, 1, BF16))
            wqc[1] = load_wchunk("wq", wq_d, 1)
            wkc[1] = load_wchunk("wk", wk_d, 1)

            # PE warm-up: throwaway matmuls on the first-arriving tiny tile
            # keep the HAM clock-gate hot while x/Wq stream in; alternate PSUM
            # banks so consecutive start/stop groups don't serialize.
            wps = pp.tile([P, S], F32, tag="pp", name="warmup_ps")
            for wi in range(10):
                off = (wi % 2) * HALF
                nc.tensor.matmul(
                    wps[:, off : off + P], ones2d[:], ones2d[:], start=True, stop=True
                )

            qT, kT = {}, {}

            def proj_qk(c, kind):
                wts = (wqc if kind == "q" else wkc)[c]
                pss = pp.tile([P, S], F32, tag="pp", name=f"ps_{kind}{c}")
                for j in range(2):
                    sl = slice(j * HALF, (j + 1) * HALF)
                    for dc in range(NCH):
                        nc.tensor.matmul(
                            pss[:, sl],
                            wts[dc][:],
                            xt[dc][:, sl],
                            start=(dc == 0),
                            stop=(dc == NCH - 1),
                        )
                o = qkpool.tile(
                    [P, S], F32R, tag="qT" if kind == "q" else "kT",
                    name=f"{kind}T_{c}",
                )
                if kind == "q":
                    nc.scalar.activation(o[:], pss[:], IDENT, bias=bq8[:, c : c + 1])
                else:
                    nc.scalar.activation(o[:], pss[:], COPY)
                (qT if kind == "q" else kT)[c] = o

            vtiles = {}

            def proj_v(sc):
                ps = pp.tile([P, S], F32, tag="pp", name=f"ps_v{sc}")
                for j in range(2):
                    sl = slice(j * HALF, (j + 1) * HALF)
                    for dc in range(NCH):
                        nc.tensor.matmul(
                            ps[:, sl],
                            xt16[dc][:, sc * P : (sc + 1) * P],
                            vh[j][dc][:],
                            start=(dc == 0),
                            stop=(dc == NCH - 1),
                        )
                vt = vpool.tile([P, H, DK + 1], BF16, tag="v", name=f"vt_{sc}")
                nc.scalar.activation(
                    vt[:, :, 0:DK], ps[:].rearrange("p (h d) -> p h d", h=H), COPY
                )
                nc.vector.memset(vt[:, :, DK : DK + 1], 1.0)
                vtiles[sc] = vt

            attn = [None, None]
            oh = [None, None]
            boe = cstpool.tile([P, D], F32, tag="boe")

            def emit_scores_exp(h, kc):
                """scores on PE, exp->bf16 on ACT, 0/1 bf16 mask mult on DVE."""
                c, r = h // 2, (h % 2) * DK
                Wc = Wd[kc]
                pss = pp.tile([P, S], F32, tag="pp", name=f"pss_{h}_{kc}")
                lhs = kT[c][r : r + DK, kc * P : (kc + 1) * P]
                p0 = min(Wc, HALF)
                nc.tensor.matmul(
                    pss[:, 0:p0], lhs, qT[c][r : r + DK, 0:p0], start=True, stop=True
                )
                if Wc > HALF:
                    nc.tensor.matmul(
                        pss[:, HALF:Wc],
                        lhs,
                        qT[c][r : r + DK, HALF:Wc],
                        start=True,
                        stop=True,
                    )
                et = exppool.tile([P, S], BF16, tag="exp", name=f"et_{h}_{kc}")
                nc.scalar.activation(et[:, 0:Wc], pss[:, 0:Wc], EXP)
                off = moff[kc]
                w = Wc - kc * P
                nc.vector.tensor_mul(
                    et[:, kc * P : Wc], et[:, kc * P : Wc], msk[:, off : off + w]
                )
                return et

            def emit_pv(h, kc, pso, et):
                Wc = Wd[kc]
                vs = vtiles[kc][:, h, :]
                p0 = min(Wc, HALF)
                nc.tensor.matmul(
                    pso[0 : DK + 1, 0:p0],
                    vs,
                    et[:, 0:p0],
                    start=(kc == NCH - 1),
                    stop=(kc == 0),
                )
                if Wc > HALF:
                    nc.tensor.matmul(
                        pso[0 : DK + 1, HALF:Wc],
                        vs,
                        et[:, HALF:Wc],
                        start=(kc == NCH - 1),
                        stop=(kc == last_b1),
                    )

            def emit_norm(h, pso):
                """Recip on DVE (PSUM->SBUF), bcast on Pool, muls on DVE.

                attn[g][e*64+d, cc, h*64+u] = O_h[u*16 + 2*(4g+cc) + e, d]/denom
                """
                rcp = rcppool.tile([1, S], F32, tag="rcp", name=f"rcp_{h}")
                nc.vector.reciprocal(rcp[:], pso[DK : DK + 1, :])
                rbc = rbcpool.tile([DK, S], F32, tag="rbc", name=f"rbc_{h}")
                nc.gpsimd.partition_broadcast(rbc[:], rcp[:])
                src = pso[0:DK, :].rearrange("d (u j) -> d j u", j=16)
                rbs = rbc[:].rearrange("d (u j) -> d j u", j=16)
                for g in range(2):
                    if attn[g] is None:
                        attn[g] = atnpool.tile(
                            [P, 4, S], BF16, tag="atn", name=f"attnq_{g}"
                        )
                    for e in range(2):
                        jsl = slice(8 * g + e, 8 * (g + 1), 2)
                        nc.vector.tensor_mul(
                            attn[g][e * DK : (e + 1) * DK, :, h * DK : (h + 1) * DK],
                            src[:, jsl, :],
                            rbs[:, jsl, :],
                        )

            def emit_oproj(sc):
                ps = po.tile([P, S], F32, tag="po", name=f"psf_{sc}")
                for j in range(2):
                    sl = slice(j * HALF, (j + 1) * HALF)
                    for cc in range(NCH):
                        nc.tensor.matmul(
                            ps[:, sl],
                            attn[cc // 4][:, cc % 4, sc * P : (sc + 1) * P],
                            oh[j][cc][:],
                            start=(cc == 0),
                            stop=(cc == NCH - 1),
                        )
                ot = osbpool.tile([P, S], F32, tag="osb", name=f"ot_{sc}")
                nc.vector.tensor_add(ot[:], ps[:], boe[:])
                nc.sync.dma_start(out_d[sc * P : (sc + 1) * P, :], ot[:])

            # ---- PE filler micro-ops: 4 matmuls each, woven between tiles ----
            def qk_fillers(c):
                """8 fillers building qT[c] then kT[c] (one j-half group of 4
                contraction steps each); pss allocated lazily per kind."""
                cell = {}

                def mk(i):
                    def run():
                        kind = "q" if i < 4 else "k"
                        j = (i % 4) // 2
                        dc0 = (i % 2) * 4
                        if i % 4 == 0:
                            cell[kind] = pp.tile(
                                [P, S], F32, tag="pp", name=f"ps_{kind}{c}"
                            )
                        ps = cell[kind]
                        wts = (wqc if kind == "q" else wkc)[c]
                        sl = slice(j * HALF, (j + 1) * HALF)
                        for dc in range(dc0, dc0 + 4):
                            nc.tensor.matmul(
                                ps[:, sl],
                                wts[dc][:],
                                xt[dc][:, sl],
                                start=(dc == 0),
                                stop=(dc == NCH - 1),
                            )
                        if i == 3:
                            o = qkpool.tile([P, S], F32R, tag="qT", name=f"qT_{c}")
                            nc.scalar.activation(
                                o[:], ps[:], IDENT, bias=bq8[:, c : c + 1]
                            )
                            qT[c] = o
                        elif i == 7:
                            o = qkpool.tile([P, S], F32R, tag="kT", name=f"kT_{c}")
                            nc.scalar.activation(o[:], ps[:], COPY)
                            kT[c] = o

                    return run

                return [mk(i) for i in range(8)]

            def oproj_fillers(sc):
                """4 fillers (j-half x cc-half) + bias/DMA on the last one."""
                cell = {}

                def mk(i):
                    def run():
                        if i == 0:
                            cell["ps"] = po.tile([P, S], F32, tag="po", name=f"psf_{sc}")
                        ps = cell["ps"]
                        j = i // 2
                        sl = slice(j * HALF, (j + 1) * HALF)
                        for cc in range((i % 2) * 4, (i % 2) * 4 + 4):
                            nc.tensor.matmul(
                                ps[:, sl],
                                attn[cc // 4][:, cc % 4, sc * P : (sc + 1) * P],
                                oh[j][cc][:],
                                start=(cc == 0),
                                stop=(cc == NCH - 1),
                            )
                        if i == 3:
                            ot = osbpool.tile([P, S], F32, tag="osb", name=f"ot_{sc}")
                            nc.vector.tensor_add(ot[:], ps[:], boe[:])
                            nc.sync.dma_start(out_d[sc * P : (sc + 1) * P, :], ot[:])

                    return run

                return [mk(i) for i in range(4)]

            # ---- the single interleaved stream ----
            from collections import deque

            pend = deque()

            def pop_pv():
                ph, pkc, ppso, pet = pend.popleft()
                emit_pv(ph, pkc, ppso, pet)
                if pkc == 0:
                    emit_norm(ph, ppso)

            proj_qk(0, "q")
            proj_qk(0, "k")
            for g in range(2):
                nc.scalar.activation(xtq16[g][:], xtq[g][:], COPY)

            for h in range(H):
                c = h // 2
                pso_cur = po.tile([P, S], F32, tag="po", name=f"pso_{h}")
                # W chunk prefetch two heads ahead of use (chunk m feeds the
                # qk fillers hosted on head 2(m-1))
                if h in (1, 2, 4, 6, 8, 10):
                    m = 2 if h == 1 else h // 2 + 2
                    if m <= NCH - 1:
                        wqc[m] = load_wchunk("wq", wq_d, m)
                        wkc[m] = load_wchunk("wk", wk_d, m)
                # filler assignment per head:
                #  h0: v tiles (descending, matches PV kc order)
                #  h1: qk(1) spread over all 8 tiles
                #  even h>=2: qk(c+1) spread over all 8 tiles
                #  odd h>=3: o_proj((h-3)//2) on tiles kc=3..0 — the psf PSUM
                #   slot only frees up at the norm(h-1) pop (tile kc=4)
                fills = {}
                if h == 1:
                    fl = qk_fillers(1)
                    fills = {7 - i: fl[i] for i in range(8)}
                elif h >= 2 and h % 2 == 0 and c + 1 <= NCH - 1:
                    fl = qk_fillers(c + 1)
                    fills = {7 - i: fl[i] for i in range(8)}
                elif h >= 3 and h % 2 == 1:
                    fl = oproj_fillers((h - 3) // 2)
                    fills = {3 - i: fl[i] for i in range(4)}
                for kc in range(NCH - 1, -1, -1):
                    et = emit_scores_exp(h, kc)
                    if len(pend) >= 4:
                        pop_pv()
                    pend.append((h, kc, pso_cur, et))
                    if h == 0:
                        # v tiles woven into head 0, descending to match PV order
                        proj_v(kc)
                        if kc == 0:
                            # Wo strips + boe: DMA queue position after wv's
                            # last use frees wpool slots
                            oh[0] = whalf("wo", wo_d, 0, BF16)
                            oh[1] = whalf("wo", wo_d, 1, BF16)
                            nc.sync.dma_start(boe[:], boe_d[:])
                    elif kc in fills:
                        fills[kc]()
            while len(pend) > 1:
                pop_pv()
            ph, pkc, ppso, pet = pend.popleft()
            emit_pv(ph, pkc, ppso, pet)
            emit_norm(ph, ppso)
            emit_oproj(NCH - 1)

    nc.compile()
    return nc


def _host_mask(prefix_b, pmax):
    """Per-core multiplicative 0/1 bf16 mask over cols [kc*128, W[kc])."""
    Wd = _widths(pmax)
    i = np.arange(P)[:, None]
    segs = []
    for kc in range(NCH):
        q = np.arange(kc * P, Wd[kc])[None, :]
        k = kc * P + i
        allowed = (q < prefix_b) | (k >= q)
        segs.append(allowed.astype(ml_dtypes.bfloat16))
    return np.concatenate(segs, axis=1)


def kernel(x, prefix, Wq, bq, Wk, bk, Wv, bv, Wo, bo, _trace=False):
    x = np.asarray(x, dtype=np.float32)
    prefix = np.asarray(prefix)
    Wq, Wk = (
        np.ascontiguousarray(np.asarray(w, np.float32)) for w in (Wq, Wk)
    )
    Wv16, Wo16 = (
        np.ascontiguousarray(np.asarray(w, np.float32).astype(ml_dtypes.bfloat16))
        for w in (Wv, Wo)
    )
    pmax = int(prefix.max())
    # Exact folds: softmax_k[(q+bq)·(k+bk)] == softmax_k[(q+bq)·k]  (q·bk and
    # bq·bk are constant over k); out = attn@Wo + (bv@Wo + bo) since sum(p)=1.
    boe = (
        np.asarray(bv, np.float64) @ np.asarray(Wo, np.float64) + np.asarray(bo)
    ).astype(np.float32)
    boe_bc = np.broadcast_to(boe.reshape(1, D), (P, D)).copy()
    bq8 = np.asarray(bq, np.float32).reshape(NCH, P).T.copy()  # [128, 8] cols
    ones2d = np.ones((P, P), dtype=np.float32)

    if pmax not in _CACHED:
        _CACHED[pmax] = build_nc(pmax)
    nc = _CACHED[pmax]

    in_maps = []
    for b in range(B):
        in_maps.append(
            {
                "xt": np.ascontiguousarray(x[b].T),
                "wq": Wq, "wk": Wk, "wv": Wv16, "wo": Wo16,
                "bq8": bq8, "boe": boe_bc, "ones2d": ones2d,
                "mskb": _host_mask(int(prefix[b]), pmax),
            }
        )

    res = run_bass_kernel_spmd(nc, in_maps, core_ids=list(range(NCORES)), trace=_trace)
    out = np.stack([res.results[b]["out"] for b in range(B)], axis=0)
    if _trace:
        return out, res
    return out


# revision 19
# speedup vs baseline: 1.0382x; 1.0380x over previous
"""Trainium2 Bass kernel for nn_MultiHeadAttention_32031866093611.

Sharding: pure data parallel — batch b -> NeuronCore b (B == n_cores == 8).
Weights replicated. No collectives.

Per-core program (batch b, S=1024, D=1024, H=16, DK=64), matmuls fp32r except
the PV stage which runs bf16 (exp output + v tiles), all PSUM accum fp32:

  qT[c] = (Wq[:, c*128:+128]).T @ xT + bq  -> [128 d', 1024 s]  (ACT Identity+bias)
  kT[c] = (Wk[:, c*128:+128]).T @ xT       -> [128 d', 1024 s]  (ACT Copy; bk is
          dropped exactly: softmax over k is invariant to the q·bk term)
  v[sc] = (xT[:, sc*128:+128]).T @ Wv      -> [128 s, 16, 64+1] bf16 (ones col;
          bv is folded on host into bo_eff = bv @ Wo + bo, exact since sum(p)=1)
  per head h (c=h//2, r=h%2*64), kc DESCENDING 7..0 with width W[kc] =
  max(max_prefix, (kc+1)*128)  (cols >= W[kc] are masked on every core):
    sT[kc] = kT[c][r:r+64, kc*128:+128].T @ qT[c][r:r+64, 0:W]   # [128 k, W q]
    eT[kc] = exp(sT[kc]) -> bf16                                  # ACT
    eT[kc][:, kc*128:W] *= mask (bf16 0/1, host-built, 4x DVE mode)
    outT  += v[kc][:, h, :].T @ eT[kc][:, 0:W]   # [65, W]; row 64 = denom
  attnT[c][r:r+64, :] = outT[0:64, :] * bcast(1/outT[64, :])
  out[sc] = (attnT[.][:, sc*128:+128]).T @ Wo + bo_eff -> [128 s, 1024 d] -> DRAM

Schedule (single in-order PE stream, PE is the binding engine at ~92% of the
kernel): per-chunk W loads let the first q/k projection start ~13us in; the v
projection tiles (descending sc, matching the descending-kc PV accumulation)
are woven between head 0's score tiles; q/k projections for chunk c+1 are
woven into head 2c+1's stream; o_proj chunk k fires two heads after head 2k+1
retires. Scores/exp/mask/PV share two PSUM score slots with the woven
projection psums (the PE never holds more than two `pp` tiles at once); PV
lags scores by 4 tiles so the in-order PE never waits on a just-issued exp.
"""

import numpy as np
import ml_dtypes

import concourse.bass as bass
import concourse.mybir as mybir
import concourse.tile as tile
from concourse import bacc
from concourse.bass_utils import run_bass_kernel_spmd

B, S, D, H = 8, 1024, 1024, 16
DK = D // H  # 64
P = 128
NCH = S // P  # 8
NCORES = 8
F32R = mybir.dt.float32r
F32 = mybir.dt.float32
BF16 = mybir.dt.bfloat16
EXP = mybir.ActivationFunctionType.Exp
IDENT = mybir.ActivationFunctionType.Identity
COPY = mybir.ActivationFunctionType.Copy
HALF = 512  # fp32 moving-operand max / one PSUM bank of fp32

_CACHED = {}


def _widths(pmax):
    """Score/exp/PV column widths per k-tile; W[7] == 1024 always."""
    return [max(pmax, (kc + 1) * P) for kc in range(NCH)]


def build_nc(pmax):
    Wd = _widths(pmax)
    mskw = [Wd[kc] - kc * P for kc in range(NCH)]
    moff = [0]
    for w in mskw[:-1]:
        moff.append(moff[-1] + w)
    msk_total = sum(mskw)
    # last-executed writer of PSUM bank 1 in the descending-kc PV group
    last_b1 = min(kc for kc in range(NCH) if Wd[kc] > HALF)

    nc = bacc.Bacc("TRN2", target_bir_lowering=False, debug=False, num_devices=NCORES)

    xt_d = nc.dram_tensor("xt", [D, S], F32R, kind="ExternalInput").ap()
    wq_d = nc.dram_tensor("wq", [D, D], F32R, kind="ExternalInput").ap()
    wk_d = nc.dram_tensor("wk", [D, D], F32R, kind="ExternalInput").ap()
    wv_d = nc.dram_tensor("wv", [D, D], BF16, kind="ExternalInput").ap()
    wo_d = nc.dram_tensor("wo", [D, D], BF16, kind="ExternalInput").ap()
    bq_d = nc.dram_tensor("bq8", [P, NCH], F32, kind="ExternalInput").ap()
    ones_d = nc.dram_tensor("ones2d", [P, P], F32R, kind="ExternalInput").ap()
    boe_d = nc.dram_tensor("boe", [P, D], F32, kind="ExternalInput").ap()
    msk_d = nc.dram_tensor("mskb", [P, msk_total], BF16, kind="ExternalInput").ap()
    out_d = nc.dram_tensor("out", [S, D], F32, kind="ExternalOutput").ap()

    with tile.TileContext(nc) as tc:
        with (
            tc.tile_pool(name="w", bufs=4) as wpool,         # wv + wo [P,8,512] halves
            tc.tile_pool(name="wqk", bufs=4) as wqkpool,     # per-chunk [P,8,P] q/k blocks
            tc.tile_pool(name="big", bufs=2) as bigpool,     # xT
            tc.tile_pool(name="atn", bufs=2) as atnpool,     # attn (own pool: xT is
            tc.tile_pool(name="qk", bufs=3) as qkpool,       #  still live at norm(0))
            tc.tile_pool(name="v", bufs=8) as vpool,
            tc.tile_pool(name="cst", bufs=1) as cstpool,
            tc.tile_pool(name="exp", bufs=5) as exppool,
            tc.tile_pool(name="rcp", bufs=2) as rcppool,
            tc.tile_pool(name="rbc", bufs=2) as rbcpool,
            tc.tile_pool(name="osb", bufs=2) as osbpool,
            tc.tile_pool(name="pp", bufs=2, space="PSUM") as pp,
            tc.tile_pool(name="po", bufs=2, space="PSUM") as po,
        ):
            # ---- DMA front: x + chunk-0 W blocks first for a fast PE start ----
            ones2d = cstpool.tile([P, P], F32R, tag="ones2d")
            nc.sync.dma_start(ones2d[:], ones_d[:])
            bq8 = cstpool.tile([P, NCH], F32, tag="bq8")
            nc.sync.dma_start(bq8[:], bq_d[:])
            xtq = [
                bigpool.tile([P, 4, S], F32R, tag="big", name=f"xtq_{g}")
                for g in range(2)
            ]

            def load_wchunk(nm, w_dram, c):
                """[128, 8, 128] tile of W columns [c*128, (c+1)*128), one DMA;
                [:, dc, :] is the contraction block for x chunk dc."""
                t = wqkpool.tile([P, NCH, P], F32R, tag="wqk", name=f"{nm}{c}")
                nc.sync.dma_start(
                    t[:],
                    w_dram[:, c * P : (c + 1) * P].rearrange(
                        "(n p) f -> p n f", p=P
                    ),
                )
                return t

            def whalf(nm, w_dram, hf, dt=F32R):
                """[128, 8, 512] tile of W columns [hf*512, (hf+1)*512), one
                DMA; [:, dc, :] is the strip for contraction chunk dc."""
                t = wpool.tile([P, NCH, HALF], dt, tag="w", name=f"{nm}{hf}")
                nc.sync.dma_start(
                    t[:],
                    w_dram[:, hf * HALF : (hf + 1) * HALF].rearrange(
                        "(n p) f -> p n f", p=P
                    ),
                )
                return t

            # x in four [128, 2, 1024] copies, W chunk-0 blocks between them,
            # so the first projection's accumulation streams with the DMA
            wqc, wkc = {}, {}
            nc.sync.dma_start(
                xtq[0][:, 0:2, :],
                xt_d[0 : 2 * P, :].rearrange("(n p) f -> p n f", p=P),
            )
            wqc[0] = load_wchunk("wq", wq_d, 0)
            nc.sync.dma_start(
                xtq[0][:, 2:4, :],
                xt_d[2 * P : 4 * P, :].rearrange("(n p) f -> p n f", p=P),
            )
            wkc[0] = load_wchunk("wk", wk_d, 0)
            nc.sync.dma_start(
                xtq[1][:, 0:2, :],
                xt_d[4 * P : 6 * P, :].rearrange("(n p) f -> p n f", p=P),
            )
            nc.sync.dma_start(
                xtq[1][:, 2:4, :],
                xt_d[6 * P : 8 * P, :].rearrange("(n p) f -> p n f", p=P),
            )
            xt = [xtq[dc // 4][:, dc % 4, :] for dc in range(NCH)]
            # bf16 copy of xT: stationary operand of the v projection (the
            # backend requires matmul operand dtypes to match; wv is bf16);
            # converted on DVE (idle during the DMA-bound start)
            xtq16 = [
                bigpool.tile([P, 4, S], BF16, tag="xt16", name=f"xtq16_{g}")
                for g in range(2)
            ]
            xt16 = [xtq16[dc // 4][:, dc % 4, :] for dc in range(NCH)]
            # mask segments for kc=5..7 land first (consumed kc-descending)
            msk = cstpool.tile([P, msk_total], BF16, tag="msk")
            nc.sync.dma_start(msk[:, moff[5] :], msk_d[:, moff[5] :])
            vh = [whalf("wv", wv_d, 0, BF16), whalf("wv", wv_d, 1, BF16)]
            nc.sync.dma_start(msk[:, 0 : moff[5]], msk_d[:, 0 : moff[5]])
            wqc[1] = load_wchunk("wq", wq_d, 1)
            wkc[1] = load_wchunk("wk", wk_d, 1)

            # PE warm-up: throwaway matmuls on the first-arriving tiny tile
            # keep the HAM clock-gate hot while x/Wq stream in; alternate PSUM
            # banks so consecutive start/stop groups don't serialize.
            wps = pp.tile([P, S], F32, tag="pp", name="warmup_ps")
            for wi in range(10):
                off = (wi % 2) * HALF
                nc.tensor.matmul(
                    wps[:, off : off + P], ones2d[:], ones2d[:], start=True, stop=True
                )

            qT, kT = {}, {}

            def proj_qk(c, kind):
                wts = (wqc if kind == "q" else wkc)[c]
                pss = pp.tile([P, S], F32, tag="pp", name=f"ps_{kind}{c}")
                for j in range(2):
                    sl = slice(j * HALF, (j + 1) * HALF)
                    for dc in range(NCH):
                        nc.tensor.matmul(
                            pss[:, sl],
                            wts[:, dc, :],
                            xt[dc][:, sl],
                            start=(dc == 0),
                            stop=(dc == NCH - 1),
                        )
                o = qkpool.tile(
                    [P, S], F32R, tag="qT" if kind == "q" else "kT",
                    name=f"{kind}T_{c}",
                )
                if kind == "q":
                    nc.scalar.activation(o[:], pss[:], IDENT, bias=bq8[:, c : c + 1])
                else:
                    nc.scalar.activation(o[:], pss[:], COPY)
                (qT if kind == "q" else kT)[c] = o

            vtiles = {}

            def proj_v(sc):
                ps = pp.tile([P, S], F32, tag="pp", name=f"ps_v{sc}")
                for j in range(2):
                    sl = slice(j * HALF, (j + 1) * HALF)
                    for dc in range(NCH):
                        nc.tensor.matmul(
                            ps[:, sl],
                            xt16[dc][:, sc * P : (sc + 1) * P],
                            vh[j][:, dc, :],
                            start=(dc == 0),
                            stop=(dc == NCH - 1),
                        )
                vt = vpool.tile([P, H, DK + 1], BF16, tag="v", name=f"vt_{sc}")
                nc.scalar.activation(
                    vt[:, :, 0:DK], ps[:].rearrange("p (h d) -> p h d", h=H), COPY
                )
                nc.vector.memset(vt[:, :, DK : DK + 1], 1.0)
                vtiles[sc] = vt

            attn = [None, None]
            oh = [None, None]
            boe = cstpool.tile([P, D], F32, tag="boe")

            def emit_scores_exp(h, kc):
                """scores on PE, exp->bf16 on ACT, 0/1 bf16 mask mult on DVE."""
                c, r = h // 2, (h % 2) * DK
                Wc = Wd[kc]
                pss = pp.tile([P, S], F32, tag="pp", name=f"pss_{h}_{kc}")
                lhs = kT[c][r : r + DK, kc * P : (kc + 1) * P]
                p0 = min(Wc, HALF)
                nc.tensor.matmul(
                    pss[:, 0:p0], lhs, qT[c][r : r + DK, 0:p0], start=True, stop=True
                )
                if Wc > HALF:
                    nc.tensor.matmul(
                        pss[:, HALF:Wc],
                        lhs,
                        qT[c][r : r + DK, HALF:Wc],
                        start=True,
                        stop=True,
                    )
                et = exppool.tile([P, S], BF16, tag="exp", name=f"et_{h}_{kc}")
                nc.scalar.activation(et[:, 0:Wc], pss[:, 0:Wc], EXP)
                off = moff[kc]
                w = Wc - kc * P
                nc.vector.tensor_mul(
                    et[:, kc * P : Wc], et[:, kc * P : Wc], msk[:, off : off + w]
                )
                return et

            def emit_pv(h, kc, pso, et):
                Wc = Wd[kc]
                vs = vtiles[kc][:, h, :]
                p0 = min(Wc, HALF)
                nc.tensor.matmul(
                    pso[0 : DK + 1, 0:p0],
                    vs,
                    et[:, 0:p0],
                    start=(kc == NCH - 1),
                    stop=(kc == 0),
                )
                if Wc > HALF:
                    nc.tensor.matmul(
                        pso[0 : DK + 1, HALF:Wc],
                        vs,
                        et[:, HALF:Wc],
                        start=(kc == NCH - 1),
                        stop=(kc == last_b1),
                    )

            def emit_norm(h, pso):
                """Recip on DVE (PSUM->SBUF), bcast on Pool, muls on DVE.

                attn[g][e*64+d, cc, h*64+u] = O_h[u*16 + 2*(4g+cc) + e, d]/denom
                """
                rcp = rcppool.tile([1, S], F32, tag="rcp", name=f"rcp_{h}")
                nc.vector.reciprocal(rcp[:], pso[DK : DK + 1, :])
                rbc = rbcpool.tile([DK, S], F32, tag="rbc", name=f"rbc_{h}")
                nc.gpsimd.partition_broadcast(rbc[:], rcp[:])
                src = pso[0:DK, :].rearrange("d (u j) -> d j u", j=16)
                rbs = rbc[:].rearrange("d (u j) -> d j u", j=16)
                for g in range(2):
                    if attn[g] is None:
                        attn[g] = atnpool.tile(
                            [P, 4, S], BF16, tag="atn", name=f"attnq_{g}"
                        )
                    for e in range(2):
                        jsl = slice(8 * g + e, 8 * (g + 1), 2)
                        nc.vector.tensor_mul(
                            attn[g][e * DK : (e + 1) * DK, :, h * DK : (h + 1) * DK],
                            src[:, jsl, :],
                            rbs[:, jsl, :],
                        )

            def emit_oproj(sc):
                ps = po.tile([P, S], F32, tag="po", name=f"psf_{sc}")
                for j in range(2):
                    sl = slice(j * HALF, (j + 1) * HALF)
                    for cc in range(NCH):
                        nc.tensor.matmul(
                            ps[:, sl],
                            attn[cc // 4][:, cc % 4, sc * P : (sc + 1) * P],
                            oh[j][:, cc, :],
                            start=(cc == 0),
                            stop=(cc == NCH - 1),
                        )
                ot = osbpool.tile([P, S], F32, tag="osb", name=f"ot_{sc}")
                nc.vector.tensor_add(ot[:], ps[:], boe[:])
                nc.sync.dma_start(out_d[sc * P : (sc + 1) * P, :], ot[:])

            # ---- PE filler micro-ops: 4 matmuls each, woven between tiles ----
            def qk_fillers(c):
                """8 fillers building qT[c] then kT[c] (one j-half group of 4
                contraction steps each); pss allocated lazily per kind."""
                cell = {}

                def mk(i):
                    def run():
                        kind = "q" if i < 4 else "k"
                        j = (i % 4) // 2
                        dc0 = (i % 2) * 4
                        if i % 4 == 0:
                            cell[kind] = pp.tile(
                                [P, S], F32, tag="pp", name=f"ps_{kind}{c}"
                            )
                        ps = cell[kind]
                        wts = (wqc if kind == "q" else wkc)[c]
                        sl = slice(j * HALF, (j + 1) * HALF)
                        for dc in range(dc0, dc0 + 4):
                            nc.tensor.matmul(
                                ps[:, sl],
                                wts[:, dc, :],
                                xt[dc][:, sl],
                                start=(dc == 0),
                                stop=(dc == NCH - 1),
                            )
                        if i == 3:
                            o = qkpool.tile([P, S], F32R, tag="qT", name=f"qT_{c}")
                            nc.scalar.activation(
                                o[:], ps[:], IDENT, bias=bq8[:, c : c + 1]
                            )
                            qT[c] = o
                        elif i == 7:
                            o = qkpool.tile([P, S], F32R, tag="kT", name=f"kT_{c}")
                            nc.scalar.activation(o[:], ps[:], COPY)
                            kT[c] = o

                    return run

                return [mk(i) for i in range(8)]

            def oproj_fillers(sc):
                """4 fillers (j-half x cc-half) + bias/DMA on the last one."""
                cell = {}

                def mk(i):
                    def run():
                        if i == 0:
                            cell["ps"] = po.tile([P, S], F32, tag="po", name=f"psf_{sc}")
                        ps = cell["ps"]
                        j = i // 2
                        sl = slice(j * HALF, (j + 1) * HALF)
                        for cc in range((i % 2) * 4, (i % 2) * 4 + 4):
                            nc.tensor.matmul(
                                ps[:, sl],
                                attn[cc // 4][:, cc % 4, sc * P : (sc + 1) * P],
                                oh[j][:, cc, :],
                                start=(cc == 0),
                                stop=(cc == NCH - 1),
                            )
                        if i == 3:
                            ot = osbpool.tile([P, S], F32, tag="osb", name=f"ot_{sc}")
                            nc.vector.tensor_add(ot[:], ps[:], boe[:])
                            nc.sync.dma_start(out_d[sc * P : (sc + 1) * P, :], ot[:])

                    return run

                return [mk(i) for i in range(4)]

            # ---- the single interleaved stream ----
            from collections import deque

            pend = deque()

            def pop_pv():
                ph, pkc, ppso, pet = pend.popleft()
                emit_pv(ph, pkc, ppso, pet)
                if pkc == 0:
                    emit_norm(ph, ppso)

            proj_qk(0, "q")
            proj_qk(0, "k")
            for g in range(2):
                for hh in range(2):
                    nc.vector.tensor_copy(
                        xtq16[g][:, 2 * hh : 2 * hh + 2, :],
                        xtq[g][:, 2 * hh : 2 * hh + 2, :],
                    )

            for h in range(H):
                c = h // 2
                pso_cur = po.tile([P, S], F32, tag="po", name=f"pso_{h}")
                # W chunk prefetch two heads ahead of use (chunk m feeds the
                # qk fillers hosted on head 2(m-1))
                if h in (1, 2, 4, 6, 8, 10):
                    m = 2 if h == 1 else h // 2 + 2
                    if m <= NCH - 1:
                        wqc[m] = load_wchunk("wq", wq_d, m)
                        wkc[m] = load_wchunk("wk", wk_d, m)
                # filler assignment per head:
                #  h0: v tiles (descending, matches PV kc order)
                #  h1: qk(1) spread over all 8 tiles
                #  even h>=2: qk(c+1) spread over all 8 tiles
                #  odd h>=3: o_proj((h-3)//2) on tiles kc=3..0 — the psf PSUM
                #   slot only frees up at the norm(h-1) pop (tile kc=4)
                fills = {}
                if h == 1:
                    fl = qk_fillers(1)
                    fills = {7 - i: fl[i] for i in range(8)}
                elif h >= 2 and h % 2 == 0 and c + 1 <= NCH - 1:
                    fl = qk_fillers(c + 1)
                    fills = {7 - i: fl[i] for i in range(8)}
                elif h >= 3 and h % 2 == 1:
                    fl = oproj_fillers((h - 3) // 2)
                    fills = {3 - i: fl[i] for i in range(4)}
                for kc in range(NCH - 1, -1, -1):
                    et = emit_scores_exp(h, kc)
                    if len(pend) >= 4:
                        pop_pv()
                    pend.append((h, kc, pso_cur, et))
                    if h == 0:
                        # v tiles woven into head 0, descending to match PV order
                        proj_v(kc)
                        if kc == 0:
                            # Wo strips + boe: DMA queue position after wv's
                            # last use frees wpool slots
                            oh[0] = whalf("wo", wo_d, 0, BF16)
                            oh[1] = whalf("wo", wo_d, 1, BF16)
                            nc.sync.dma_start(boe[:], boe_d[:])
                    elif kc in fills:
                        fills[kc]()
            while len(pend) > 1:
                pop_pv()
            ph, pkc, ppso, pet = pend.popleft()
            emit_pv(ph, pkc, ppso, pet)
            emit_norm(ph, ppso)
            emit_oproj(NCH - 1)

    nc.compile()
    return nc


def _host_mask(prefix_b, pmax):
    """Per-core multiplicative 0/1 bf16 mask over cols [kc*128, W[kc])."""
    Wd = _widths(pmax)
    i = np.arange(P)[:, None]
    segs = []
    for kc in range(NCH):
        q = np.arange(kc * P, Wd[kc])[None, :]
        k = kc * P + i
        allowed = (q < prefix_b) | (k >= q)
        segs.append(allowed.astype(ml_dtypes.bfloat16))
    return np.concatenate(segs, axis=1)


def kernel(x, prefix, Wq, bq, Wk, bk, Wv, bv, Wo, bo, _trace=False):
    x = np.asarray(x, dtype=np.float32)
    prefix = np.asarray(prefix)
    Wq, Wk = (
        np.ascontiguousarray(np.asarray(w, np.float32)) for w in (Wq, Wk)
    )
    Wv16, Wo16 = (
        np.ascontiguousarray(np.asarray(w, np.float32).astype(ml_dtypes.bfloat16))
        for w in (Wv, Wo)
    )
    pmax = int(prefix.max())
    # Exact folds: softmax_k[(q+bq)·(k+bk)] == softmax_k[(q+bq)·k]  (q·bk and
    # bq·bk are constant over k); out = attn@Wo + (bv@Wo + bo) since sum(p)=1.
    boe = (
        np.asarray(bv, np.float64) @ np.asarray(Wo, np.float64) + np.asarray(bo)
    ).astype(np.float32)
    boe_bc = np.broadcast_to(boe.reshape(1, D), (P, D)).copy()
    bq8 = np.asarray(bq, np.float32).reshape(NCH, P).T.copy()  # [128, 8] cols
    ones2d = np.ones((P, P), dtype=np.float32)

    if pmax not in _CACHED:
        _CACHED[pmax] = build_nc(pmax)
    nc = _CACHED[pmax]

    in_maps = []
    for b in range(B):
        in_maps.append(
            {
                "xt": np.ascontiguousarray(x[b].T),
                "wq": Wq, "wk": Wk, "wv": Wv16, "wo": Wo16,
                "bq8": bq8, "boe": boe_bc, "ones2d": ones2d,
                "mskb": _host_mask(int(prefix[b]), pmax),
            }
        )

    res = run_bass_kernel_spmd(nc, in_maps, core_ids=list(range(NCORES)), trace=_trace)
    out = np.stack([res.results[b]["out"] for b in range(B)], axis=0)
    if _trace:
        return out, res
    return out
